# Initial kernel scaffold
#
"""AlgebraicTransition kernel for 8 TRN2 NeuronCores.

out[b] = blockdiag-matmul( state_embedding[b].reshape(16,32,32),
                           Mn[:, transitions[input_symbols[b]]] )
with Mn = reps / (frobenius_norm + 1e-6).

Strategy: pure data parallel over batch. Host computes the symbol->element
routing, deals batch rows round-robin per element-group to the 8 cores
(padding each group to a multiple of 8 with duplicate rows so every core
sees an identical group structure -> one SPMD Bass program), and
pre-transposes the embedding into the layout the TensorEngine needs.
Device: per 512-col PSUM piece, 4 concurrent 32x32 tile_position matmuls
per group-chunk contract over j; the DVE 32x32 block transpose that
evacuates PSUM back to SBUF doubles as the output transpose.
"""

import os
import sys

sys.path.insert(0, "/opt/trn_rl_repo")

import numpy as np

import concourse.bass as bass
import concourse.mybir as mybir
from concourse import tile
from concourse.bass_utils import run_bass_kernel_spmd

NCORES = 8
B = 4096
NR = 16          # reps
D = 32           # block dim
NPACK = 4        # reps per pack (K = 4*32 = 128 partitions)
EMB = NR * D * D
F32 = mybir.dt.float32

TILE_ROWS = 64   # rows per SBUF tile (64*32 = 2048 cols, 1 MiB)
PIECE_ROWS = 16  # rows per PSUM piece (16*32 = 512 cols, one bank)

# Stash of the last run's BassKernelResults (exec_time_ns etc.) for test.py.
last_results = None


def _build_program(R, chunks, n_e):
    """One SPMD program; identical structure on every core.

    chunks: list of (row_start, row_end, e_idx) covering [0, R), each fully
    inside one 16-row piece.
    """
    nc = bass.Bass(target_bir_lowering=False)
    emb_d = nc.declare_dram_parameter("emb_t", [NPACK, 128, R * D], F32, False)
    w_d = nc.declare_dram_parameter("w", [128, n_e * NPACK * D], F32, False)
    out_d = nc.declare_dram_parameter("out", [NPACK, 128, R * D], F32, True)

    with tile.TileContext(nc) as tc:
        with (
            tc.tile_pool(name="wpool", bufs=1) as wpool,
            tc.tile_pool(name="inpool", bufs=3) as inpool,
            tc.tile_pool(name="outpool", bufs=3) as outpool,
            tc.tile_pool(name="psum", bufs=8, space="PSUM") as psumpool,
        ):
            w_t = wpool.tile([128, n_e * NPACK * D], F32)
            nc.sync.dma_start(out=w_t[:, :], in_=w_d[:, :])

            for p in range(NPACK):
                for row0 in range(0, R, TILE_ROWS):
                    row1 = min(row0 + TILE_ROWS, R)
                    w = (row1 - row0) * D
                    in_t = inpool.tile([128, TILE_ROWS * D], F32, tag="in")
                    nc.sync.dma_start(
                        out=in_t[:, :w], in_=emb_d[p, :, row0 * D : row1 * D]
                    )
                    out_t = outpool.tile([128, TILE_ROWS * D], F32, tag="out")
                    for p0 in range(row0, row1, PIECE_ROWS):
                        p1 = min(p0 + PIECE_ROWS, row1)
                        pw = (p1 - p0) * D
                        ps = psumpool.tile([128, PIECE_ROWS * D], F32, tag="ps")
                        for (a, b, e) in chunks:
                            if a >= p1 or b <= p0:
                                continue
                            assert a >= p0 and b <= p1
                            c0, c1 = (a - p0) * D, (b - p0) * D
                            r0, r1 = (a - row0) * D, (b - row0) * D
                            wcol = (e * NPACK + p) * D
                            for rp in range(4):
                                q0 = 32 * rp
                                nc.tensor.matmul(
                                    ps[q0 : q0 + 32, c0:c1],
                                    w_t[q0 : q0 + 32, wcol : wcol + D],
                                    in_t[q0 : q0 + 32, r0:r1],
                                    start=True,
                                    stop=True,
                                    tile_position=(q0, q0),
                                )
                        nc.vector.transpose(
                            out_t[:, (p0 - row0) * D : (p1 - row0) * D], ps[:, :pw]
                        )
                    nc.sync.dma_start(
                        out=out_d[p, :, row0 * D : row1 * D], in_=out_t[:, :w]
                    )
    return nc


def kernel(state_embedding, input_symbols, reps, transitions):
    global last_results
    emb = np.ascontiguousarray(np.asarray(state_embedding, dtype=np.float32))
    syms = np.asarray(input_symbols).astype(np.int64)
    reps = np.asarray(reps, dtype=np.float32)
    trans = np.asarray(transitions).astype(np.int64)

    # --- host routing ---------------------------------------------------
    t = trans[syms]                                   # [B] element per row
    elems, counts = np.unique(t, return_counts=True)  # used elements, sorted
    n_e = len(elems)
    order = np.argsort(t, kind="stable")

    per_core = []   # per group: [c_e, NCORES] row indices (padded w/ dups)
    pos = 0
    for g in range(n_e):
        rows = order[pos : pos + counts[g]]
        pos += counts[g]
        c_e = -(-len(rows) // NCORES)                 # ceil
        padded = np.concatenate(
            [rows, np.full(c_e * NCORES - len(rows), rows[0], dtype=rows.dtype)]
        )
        per_core.append(padded.reshape(c_e, NCORES))
    idx_all = np.concatenate(per_core, axis=0).T      # [NCORES, R]
    R = idx_all.shape[1]
    counts_core = np.array([pc.shape[0] for pc in per_core])

    # chunk structure: split each group's row-range at 16-row grid lines
    bounds = np.concatenate([[0], np.cumsum(counts_core)])
    chunks = []
    for g in range(n_e):
        a = bounds[g]
        while a < bounds[g + 1]:
            b = min((a // PIECE_ROWS + 1) * PIECE_ROWS, bounds[g + 1])
            chunks.append((int(a), int(b), g))
            a = b

    # --- host data prep -------------------------------------------------
    # emb_t[core][p][rp*32+j][b*32+i] = emb[row, (4p+rp)*1024 + i*32 + j]
    gathered = emb[idx_all.reshape(-1)].reshape(NCORES, R, NPACK, 4, D, D)
    emb_t = np.ascontiguousarray(
        gathered.transpose(0, 2, 3, 5, 1, 4)
    ).reshape(NCORES, NPACK, 128, R * D)

    # normalized rep matrices, laid out for direct SBUF load
    # w[rp*32+j, (e*4+p)*32+k] = Mn[4p+rp, elems[e], j, k]
    sel = reps[:, elems].astype(np.float64)           # [16, n_e, 32, 32]
    fro = np.sqrt(np.sum(sel * sel, axis=(-2, -1), keepdims=True))
    mn = (sel / (fro + 1e-6)).astype(np.float32)
    w_host = np.ascontiguousarray(
        mn.reshape(NPACK, 4, n_e, D, D).transpose(1, 3, 2, 0, 4)
    ).reshape(128, n_e * NPACK * D)

    # --- device ---------------------------------------------------------
    nc = _build_program(R, chunks, n_e)
    in_maps = [{"emb_t": emb_t[k], "w": w_host} for k in range(NCORES)]
    try:
        res = run_bass_kernel_spmd(nc, in_maps, core_ids=list(range(NCORES)))
    except Exception:
        if os.environ.get("BASS_TRACE"):
            os.environ["BASS_NEVER_TRACE"] = "1"
            res = run_bass_kernel_spmd(nc, in_maps, core_ids=list(range(NCORES)))
            del os.environ["BASS_NEVER_TRACE"]
        else:
            raise
    last_results = res

    # --- host unpack ----------------------------------------------------
    out_full = np.empty((B, EMB), dtype=np.float32)
    for k in range(NCORES):
        dev = res.results[k]["out"].reshape(NPACK, 4, D, R, D)  # p, rp, i, b, k
        rows = np.ascontiguousarray(dev.transpose(3, 0, 1, 2, 4)).reshape(R, EMB)
        out_full[idx_all[k]] = rows
    return out_full


# revision 24
# speedup vs baseline: 1.0266x; 1.0266x over previous
"""AlgebraicTransition kernel for 8 TRN2 NeuronCores.

out[b] = blockdiag-matmul( state_embedding[b].reshape(16,32,32),
                           Mn[:, transitions[input_symbols[b]]] )
with Mn = reps / (frobenius_norm + 1e-6).

Strategy: pure data parallel over batch. Host computes the symbol->element
routing, deals batch rows round-robin per element-group to the 8 cores
(padding each group to a multiple of 8 with duplicate rows so every core
sees an identical group structure -> one SPMD Bass program), and
pre-transposes the embedding into the layout the TensorEngine needs.
Device: weights live in SBUF as per-(element,pack) 128x128 block-diagonal
lhsT tiles (built once by memset + 4 scatter DMAs of the compact table);
each group-chunk is one K=128 matmul; the DVE 32x32 block transpose that
evacuates PSUM back to SBUF doubles as the output transpose.
"""

import os
import sys

sys.path.insert(0, "/opt/trn_rl_repo")

import numpy as np

import concourse.bacc as bacc
import concourse.mybir as mybir
from concourse import tile
from concourse.bass_utils import run_bass_kernel_spmd

NCORES = 8
B = 4096
NR = 16          # reps
D = 32           # block dim
NPACK = 4        # reps per pack (K = 4*32 = 128 partitions)
EMB = NR * D * D
F32 = mybir.dt.float32
# fp32 matmul costs 4 cyc/col (2 half-speed passes); float32r streams at
# 1 cyc/col for N>=256 with relaxed internal precision. Both need dst
# partition base 0, which the block-diagonal lhsT layout provides.
MM_DT = mybir.dt.float32r

TILE_ROWS = 64   # max rows per SBUF tile (64*32 = 2048 cols, 1 MiB)
PIECE_ROWS = 16  # max rows per PSUM piece (16*32 = 512 cols, one bank)

# Stash of the last run's BassKernelResults (exec_time_ns etc.) for test.py.
last_results = None


def _layout(counts_core):
    """Fixed-grid chunk/piece/tile structure from per-core group counts.

    chunks: (a, b, e) — one matmul each, split at group bounds and the
    16-row PSUM grid. pieces: fixed 16-row PSUM banks. tiles: fixed
    64-row DMA tiles.
    """
    R = sum(counts_core)
    bounds = np.concatenate([[0], np.cumsum(counts_core)])
    chunks = []
    for g in range(len(counts_core)):
        a = bounds[g]
        while a < bounds[g + 1]:
            b = min((a // PIECE_ROWS + 1) * PIECE_ROWS, bounds[g + 1])
            chunks.append((int(a), int(b), g))
            a = b

    tiles = []
    for ta in range(0, R, TILE_ROWS):
        tb = min(ta + TILE_ROWS, R)
        pieces = []
        for pa in range(ta, tb, PIECE_ROWS):
            pb = min(pa + PIECE_ROWS, tb)
            pcs = [c for c in chunks if c[0] < pb and c[1] > pa]
            pieces.append((pa, pb, pcs))
        tiles.append((ta, tb, pieces))
    return tiles


def _build_program(R, tiles, n_e):
    nc = bacc.Bacc(None, target_bir_lowering=False)
    emb_d = nc.declare_dram_parameter("emb_t", [NPACK, 128, R * D], MM_DT, False)
    w_d = nc.declare_dram_parameter("w", [NPACK, n_e, NPACK, D, D], MM_DT, False)
    out_d = nc.declare_dram_parameter("out", [NPACK, 128, R * D], F32, True)

    with tile.TileContext(nc) as tc:
        with (
            tc.tile_pool(name="wpool", bufs=1) as wpool,
            tc.tile_pool(name="inpool", bufs=4) as inpool,
            tc.tile_pool(name="outpool", bufs=4) as outpool,
            tc.tile_pool(name="psum", bufs=8, space="PSUM") as psumpool,
        ):
            # block-diagonal lhsT region: one 128x128 block per (pack, e),
            # pack-major so pack p's strip is ready before its matmuls
            w_t = wpool.tile([128, n_e * NPACK * 128], MM_DT)
            for p in range(NPACK):
                strip = w_t[:, p * n_e * 128 : (p + 1) * n_e * 128]
                nc.vector.memset(strip.bitcast(F32), 0.0)
                for rp in range(NPACK):
                    q0 = 32 * rp
                    dest = strip[q0 : q0 + 32, :].rearrange(
                        "q (e c) -> q e c", c=128
                    )[:, :, q0 : q0 + 32]
                    src = w_d[rp, :, p].rearrange("e j k -> j e k")
                    nc.sync.dma_start(out=dest, in_=src)

            for p in range(NPACK):
                for (ta, tb, pieces) in tiles:
                    w = (tb - ta) * D
                    in_t = inpool.tile([128, TILE_ROWS * D], MM_DT, tag="in")
                    nc.sync.dma_start(
                        out=in_t[:, :w], in_=emb_d[p, :, ta * D : tb * D]
                    )
                    out_t = outpool.tile([128, TILE_ROWS * D], F32, tag="out")
                    for (pa, pb, chunks) in pieces:
                        pw = (pb - pa) * D
                        ps = psumpool.tile([128, PIECE_ROWS * D], F32, tag="ps")
                        for (a, b, e) in chunks:
                            blk = (p * n_e + e) * 128
                            nc.tensor.matmul(
                                ps[:, (a - pa) * D : (b - pa) * D],
                                w_t[:, blk : blk + 128],
                                in_t[:, (a - ta) * D : (b - ta) * D],
                                start=True,
                                stop=True,
                            )
                        nc.vector.transpose(
                            out_t[:, (pa - ta) * D : (pb - ta) * D], ps[:, :pw]
                        )
                    nc.sync.dma_start(
                        out=out_d[p, :, ta * D : tb * D], in_=out_t[:, :w]
                    )
    nc.compile()
    return nc


def kernel(state_embedding, input_symbols, reps, transitions):
    global last_results
    emb = np.ascontiguousarray(np.asarray(state_embedding, dtype=np.float32))
    syms = np.asarray(input_symbols).astype(np.int64)
    reps = np.asarray(reps, dtype=np.float32)
    trans = np.asarray(transitions).astype(np.int64)

    # --- host routing ---------------------------------------------------
    t = trans[syms]                                   # [B] element per row
    elems, counts = np.unique(t, return_counts=True)  # used elements, sorted
    n_e = len(elems)
    order = np.argsort(t, kind="stable")

    per_core = []   # per group: [c_e, NCORES] row indices (padded w/ dups)
    pos = 0
    for g in range(n_e):
        rows = order[pos : pos + counts[g]]
        pos += counts[g]
        c_e = -(-len(rows) // NCORES)                 # ceil
        padded = np.concatenate(
            [rows, np.full(c_e * NCORES - len(rows), rows[0], dtype=rows.dtype)]
        )
        per_core.append(padded.reshape(c_e, NCORES))
    idx_all = np.concatenate(per_core, axis=0).T      # [NCORES, R]
    R = idx_all.shape[1]
    counts_core = [pc.shape[0] for pc in per_core]
    tiles = _layout(counts_core)

    # --- host data prep -------------------------------------------------
    # emb_t[core][p][rp*32+j][b*32+i] = emb[row, (4p+rp)*1024 + i*32 + j]
    gathered = emb[idx_all.reshape(-1)].reshape(NCORES, R, NPACK, 4, D, D)
    emb_t = np.ascontiguousarray(
        gathered.transpose(0, 2, 3, 5, 1, 4)
    ).reshape(NCORES, NPACK, 128, R * D)

    # normalized rep matrices, compact: w[rp, e, p, j, k] = Mn[4p+rp, elems[e]]
    sel = reps[:, elems].astype(np.float64)           # [16, n_e, 32, 32]
    fro = np.sqrt(np.sum(sel * sel, axis=(-2, -1), keepdims=True))
    mn = (sel / (fro + 1e-6)).astype(np.float32)      # [16, n_e, 32, 32]
    w_host = np.ascontiguousarray(
        mn.reshape(NPACK, 4, n_e, D, D).transpose(1, 2, 0, 3, 4)
    )                                                 # [rp, e, p, D, D]

    # --- device ---------------------------------------------------------
    nc = _build_program(R, tiles, n_e)
    in_maps = [{"emb_t": emb_t[k], "w": w_host} for k in range(NCORES)]
    try:
        res = run_bass_kernel_spmd(nc, in_maps, core_ids=list(range(NCORES)))
    except Exception:
        if os.environ.get("BASS_TRACE"):
            os.environ["BASS_NEVER_TRACE"] = "1"
            res = run_bass_kernel_spmd(nc, in_maps, core_ids=list(range(NCORES)))
            del os.environ["BASS_NEVER_TRACE"]
        else:
            raise
    last_results = res

    # --- host unpack ----------------------------------------------------
    out_full = np.empty((B, EMB), dtype=np.float32)
    for k in range(NCORES):
        dev = res.results[k]["out"].reshape(NPACK, 4, D, R, D)  # p, rp, i, b, k
        rows = np.ascontiguousarray(dev.transpose(3, 0, 1, 2, 4)).reshape(R, EMB)
        out_full[idx_all[k]] = rows
    return out_full
